# revision 8
# baseline (speedup 1.0000x reference)
"""Bahdanau additive attention on 8 Trainium2 NeuronCores.

Problem: B=32, S=1024, H=1024 fp32.
  U_h   = dec @ U_w.T                  [B, H]
  W_s   = enc @ W_w.T                  [B, S, H]
  att   = tanh(U_h[:,None,:] + W_s) @ v  [B, S]
  alpha = softmax(att, axis=1)
  ctx   = einsum('bs,bsh->bh', alpha, enc)

Sharding: data-parallel over B across 8 cores (4 batches per core),
U_w / W_w / v_w replicated.

Per-core kernel (all matmuls in bf16 with fp32 PSUM accumulation):
  - enc streamed in [128,1024] chunks, cast to bf16 (kept as encN for the
    context matmul), PE-transposed (bf16, via identity) into encT [h,s].
  - W_sT tiles [o=128, s=512] = W_wT.T @ encT accumulated over 8 h-tiles.
  - ScalarE tanh with per-partition bias U_hT[o] fused on PSUM evacuation.
  - score reduction over o as a PE matvec with v (accumulated over o-tiles).
  - batched softmax over the 4 local batches on VectorE/ScalarE.
  - alpha PE-transposed; context = alphaT.T @ encN accumulated over s-tiles.
"""

import numpy as np
from contextlib import ExitStack

import concourse.bacc as bacc
import concourse.mybir as mybir
import concourse.tile as tile
from concourse import masks
from concourse.bass_utils import run_bass_kernel_spmd

N_CORES = 8
B = 32
B_L = B // N_CORES  # 4 batches per core
S = 1024
H = 1024
P = 128
NT = 8  # 1024 / 128 tiles
F32 = mybir.dt.float32
BF16 = mybir.dt.bfloat16
AF = mybir.ActivationFunctionType


def _emit(tc, stop_after="full"):
    nc = tc.nc
    dec = nc.dram_tensor("dec", [B_L, H], F32, kind="ExternalInput").ap()
    enc = nc.dram_tensor("enc", [B_L, S, H], F32, kind="ExternalInput").ap()
    U_w = nc.dram_tensor("U_w", [H, H], F32, kind="ExternalInput").ap()
    W_w = nc.dram_tensor("W_w", [H, H], F32, kind="ExternalInput").ap()
    v_w = nc.dram_tensor("v_w", [H], F32, kind="ExternalInput").ap()
    ctx_out = nc.dram_tensor("ctx", [B_L, H], F32, kind="ExternalOutput").ap()
    alpha_out = nc.dram_tensor("alpha", [B_L, S], F32, kind="ExternalOutput").ap()

    ctx = ExitStack()
    const = ctx.enter_context(tc.tile_pool(name="const", bufs=1))
    natp = ctx.enter_context(tc.tile_pool(name="nat", bufs=3))
    encTp = ctx.enter_context(tc.tile_pool(name="encT", bufs=2))
    tanhp = ctx.enter_context(tc.tile_pool(name="tanh", bufs=3))
    mmps = ctx.enter_context(tc.tile_pool(name="mmps", bufs=4, space="PSUM"))
    attps = ctx.enter_context(tc.tile_pool(name="attps", bufs=1, space="PSUM"))
    trps = ctx.enter_context(tc.tile_pool(name="trps", bufs=2, space="PSUM"))

    ident = const.tile([P, P], BF16)
    identf = const.tile([P, P], F32)
    masks.make_identity(nc, ident[:])
    masks.make_identity(nc, identf[:])

    # --- small constants: v (transposed) and dec (transposed), cast bf16 ---
    vT_f = const.tile([P, NT], F32)
    nc.sync.dma_start(vT_f[:], v_w.rearrange("(t p) -> p t", p=P))
    vT = const.tile([P, NT], BF16)
    nc.vector.tensor_copy(vT[:], vT_f[:])

    dec_nat = const.tile([B_L, H], F32)
    nc.sync.dma_start(dec_nat[:], dec[:])
    decT = const.tile([P, NT, B_L], BF16)
    for k in range(NT):
        ps = mmps.tile([P, B_L], F32, tag="mm", name="dec_ps")
        nc.tensor.transpose(
            ps[:], dec_nat[:, k * P : (k + 1) * P], identf[0:B_L, 0:B_L]
        )
        nc.vector.tensor_copy(decT[:, k, :], ps[:])

    # --- weight transposes: W_wT / U_wT in bf16, [h_in, h_tile, out] ---
    W_wT = const.tile([P, NT, H], BF16)
    U_wT = encTp.tile([P, NT, H], BF16, tag="encT")

    def transpose_in(dst, src_dram):
        # src [1024, 1024] row-major -> dst[p, j, o] = src[o, 128*j + p]
        for i in range(NT):  # row tile of src (o)
            natt = natp.tile([P, H], F32, tag="nat")
            nc.sync.dma_start(natt[:], src_dram[i * P : (i + 1) * P, :])
            natb = natp.tile([P, H], BF16, tag="natb")
            nc.vector.tensor_copy(natb[:], natt[:])
            for g in range(2):  # groups of 4 h-tiles
                ps = trps.tile([P, 4, P], BF16, tag="tr")
                for jj in range(4):
                    j = 4 * g + jj
                    nc.tensor.transpose(
                        ps[:, jj, :], natb[:, j * P : (j + 1) * P], ident[:]
                    )
                nc.vector.tensor_copy(
                    dst[:, 4 * g : 4 * g + 4, i * P : (i + 1) * P], ps[:]
                )

    transpose_in(W_wT, W_w)
    transpose_in(U_wT, U_w)

    # --- U_hT[o, b] = sum_h U_wT[h, o].T @ decT[h, b], per o-tile ---
    U_hT = const.tile([P, NT, B_L], F32)
    for i in range(NT):
        ps = mmps.tile([P, B_L], F32, tag="mm")
        for j in range(NT):
            nc.tensor.matmul(
                ps[:],
                U_wT[:, j, i * P : (i + 1) * P],
                decT[:, j, :],
                start=(j == 0),
                stop=(j == NT - 1),
            )
        nc.vector.tensor_copy(U_hT[:, i, :], ps[:])

    # --- persistent bf16 natural-layout copy of enc (for the context matmul)
    encN = const.tile([P, B_L, NT, H], BF16)  # [s_in, b, s_tile, h]
    att_stage = const.tile([1, B_L * S], F32)

    def emit_chunk_load(b, k):
        """DMA enc[b, s-tile k] chunk, cast to bf16 into encN."""
        natt = natp.tile([P, H], F32, tag="nat")
        nc.sync.dma_start(natt[:], enc[b, k * P : (k + 1) * P, :])
        nc.vector.tensor_copy(encN[:, b, k, :], natt[:])

    def emit_transpose_group(encT_b, b, k, g):
        """PE-transpose 4 h-tiles of chunk (b, k) into encT_b."""
        ps = trps.tile([P, 4, P], BF16, tag="tr")
        for jj in range(4):
            j = 4 * g + jj
            nc.tensor.transpose(
                ps[:, jj, :], encN[:, b, k, j * P : (j + 1) * P], ident[:]
            )
        nc.vector.tensor_copy(
            encT_b[:, 4 * g : 4 * g + 4, k * P : (k + 1) * P], ps[:]
        )

    if stop_after == "setup":
        dbg = const.tile([1, B_L * S], F32)
        nc.vector.tensor_copy(dbg[0:1, 0 : NT * B_L], U_hT.rearrange("p t b -> p (t b)")[0:1, :])
        nc.sync.dma_start(alpha_out.rearrange("b s -> (b s)"), dbg[0:1, :])
        nc.gpsimd.memset(dbg[:], 0.0)
        nc.sync.dma_start(ctx_out.rearrange("b h -> (b h)"), dbg[0:1, :])
        ctx.close()
        return

    # Pre-load + transpose batch 0
    encT_cur = encTp.tile([P, NT, S], BF16, tag="encT")
    for k in range(NT):
        emit_chunk_load(0, k)
    for k in range(NT):
        for g in range(2):
            emit_transpose_group(encT_cur, 0, k, g)

    for b in range(B_L):
        # next batch: loads emitted up-front; transposes interleaved below
        encT_next = None
        if b + 1 < B_L:
            encT_next = encTp.tile([P, NT, S], BF16, tag="encT")
            for k in range(NT):
                emit_chunk_load(b + 1, k)

        att_ps = [attps.tile([P, 512], F32, tag=f"att{c}", name=f"att_ps{c}") for c in range(2)]
        tanh_prev = None
        for i in range(NT):
            ps = [mmps.tile([P, 512], F32, tag="mm", name=f"mm_ps{c2}") for c2 in range(2)]
            for j in range(NT):
                lhsT = W_wT[:, j, i * P : (i + 1) * P]
                for c in range(2):
                    nc.tensor.matmul(
                        ps[c][:],
                        lhsT,
                        encT_cur[:, j, c * 512 : (c + 1) * 512],
                        start=(j == 0),
                        stop=(j == NT - 1),
                    )
            # v-matvec for previous o-tile (tanh ready by now; keeps PE rolling)
            if tanh_prev is not None:
                ip, th = tanh_prev
                for c in range(2):
                    nc.tensor.matmul(
                        att_ps[c][0:1, :],
                        vT[:, ip : ip + 1],
                        th[:, c * 512 : (c + 1) * 512],
                        start=(ip == 0),
                        stop=(ip == NT - 1),
                    )
            # interleave next batch's transposes into PE stream
            if encT_next is not None:
                for g in range(2):
                    emit_transpose_group(encT_next, b + 1, i, g)
            th = tanhp.tile([P, 1024], BF16, tag="tanh")
            for c in range(2):
                nc.scalar.activation(
                    th[:, c * 512 : (c + 1) * 512],
                    ps[c][:],
                    AF.Tanh,
                    bias=U_hT[:, i, b : b + 1],
                    scale=1.0,
                )
            tanh_prev = (i, th)

        ip, th = tanh_prev
        for c in range(2):
            nc.tensor.matmul(
                att_ps[c][0:1, :],
                vT[:, ip : ip + 1],
                th[:, c * 512 : (c + 1) * 512],
                start=(ip == 0),
                stop=(ip == NT - 1),
            )
        for c in range(2):
            nc.vector.tensor_copy(
                att_stage[0:1, b * S + c * 512 : b * S + (c + 1) * 512],
                att_ps[c][0:1, :],
            )
        if encT_next is not None:
            encT_cur = encT_next

    if stop_after == "phase1":
        nc.sync.dma_start(alpha_out.rearrange("b s -> (b s)"), att_stage[0:1, :])
        dbg = const.tile([1, B_L * H], F32)
        nc.gpsimd.memset(dbg[:], 0.0)
        nc.sync.dma_start(ctx_out.rearrange("b h -> (b h)"), dbg[0:1, :])
        ctx.close()
        return

    # --- softmax over s, per batch, on the partition-0 stage row ---
    smax = const.tile([1, B_L], F32)
    negmax = const.tile([1, B_L], F32)
    ssum = const.tile([1, B_L], F32)
    srec = const.tile([1, B_L], F32)
    exp_stage = const.tile([1, B_L * S], F32)
    for b in range(B_L):
        seg = att_stage[0:1, b * S : (b + 1) * S]
        nc.vector.reduce_max(smax[0:1, b : b + 1], seg, axis=mybir.AxisListType.X)
        nc.vector.tensor_scalar_mul(negmax[0:1, b : b + 1], smax[0:1, b : b + 1], -1.0)
        nc.scalar.activation(
            exp_stage[0:1, b * S : (b + 1) * S],
            seg,
            AF.Exp,
            bias=negmax[0:1, b : b + 1],
            scale=1.0,
        )
        nc.vector.reduce_sum(
            ssum[0:1, b : b + 1],
            exp_stage[0:1, b * S : (b + 1) * S],
            axis=mybir.AxisListType.X,
        )
    nc.vector.reciprocal(srec[:], ssum[:])
    alpha_stage = const.tile([1, B_L * S], F32)
    for b in range(B_L):
        nc.vector.tensor_scalar_mul(
            alpha_stage[0:1, b * S : (b + 1) * S],
            exp_stage[0:1, b * S : (b + 1) * S],
            srec[0:1, b : b + 1],
        )
    # write alpha output; also serves as the DRAM bounce for transposition
    nc.sync.dma_start(alpha_out.rearrange("b s -> (b s)"), alpha_stage[0:1, :])

    if stop_after == "phase2":
        dbg = const.tile([1, B_L * H], F32)
        nc.gpsimd.memset(dbg[:], 0.0)
        nc.sync.dma_start(ctx_out.rearrange("b h -> (b h)"), dbg[0:1, :])
        ctx.close()
        return

    # --- alphaT[s_in, s_tile, b] via strided read-back of alpha_out ---
    alphaT_f = const.tile([P, NT, B_L], F32)
    for b in range(B_L):
        nc.sync.dma_start(
            alphaT_f[:, :, b], alpha_out[b].rearrange("(k p) -> p k", p=P)
        )
    alphaT = const.tile([P, NT, B_L], BF16)
    nc.vector.tensor_copy(alphaT[:], alphaT_f[:])

    # --- context: ctx[b, h] = sum_s alpha[b, s] * enc[b, s, h] ---
    ctx_stage = const.tile([1, B_L * H], F32)
    for b in range(B_L):
        ps = [mmps.tile([P, 512], F32, tag="mm", name=f"mm_ps{c2}") for c2 in range(2)]
        for k in range(NT):
            lhsT = alphaT[:, k, b : b + 1]
            for c in range(2):
                nc.tensor.matmul(
                    ps[c][0:1, :],
                    lhsT,
                    encN[:, b, k, c * 512 : (c + 1) * 512],
                    start=(k == 0),
                    stop=(k == NT - 1),
                )
        for c in range(2):
            nc.vector.tensor_copy(
                ctx_stage[0:1, b * H + c * 512 : b * H + (c + 1) * 512],
                ps[c][0:1, :],
            )
    nc.sync.dma_start(
        ctx_out.rearrange("b h -> (b h)"), ctx_stage[0:1, :]
    )
    ctx.close()


_CACHED = None


def _build(stop_after="full"):
    global _CACHED
    if _CACHED is None:
        nc = bacc.Bacc("TRN2", target_bir_lowering=False, debug=False)
        with tile.TileContext(nc) as tc:
            _emit(tc, stop_after=stop_after)
        nc.compile()
        _CACHED = nc
    return _CACHED


def kernel(
    decoder_hidden: np.ndarray,
    encoder_outputs: np.ndarray,
    U_w: np.ndarray,
    W_w: np.ndarray,
    v_w: np.ndarray,
):
    dec = np.ascontiguousarray(np.asarray(decoder_hidden, dtype=np.float32))
    enc = np.ascontiguousarray(np.asarray(encoder_outputs, dtype=np.float32))
    U = np.ascontiguousarray(np.asarray(U_w, dtype=np.float32))
    W = np.ascontiguousarray(np.asarray(W_w, dtype=np.float32))
    v = np.ascontiguousarray(np.asarray(v_w, dtype=np.float32))

    nc = _build()
    in_maps = []
    for c in range(N_CORES):
        sl = slice(c * B_L, (c + 1) * B_L)
        in_maps.append(
            {"dec": dec[sl], "enc": enc[sl], "U_w": U, "W_w": W, "v_w": v}
        )
    res = run_bass_kernel_spmd(nc, in_maps, core_ids=list(range(N_CORES)))
    context = np.concatenate([res.results[c]["ctx"] for c in range(N_CORES)], axis=0)
    alpha = np.concatenate([res.results[c]["alpha"] for c in range(N_CORES)], axis=0)
    return (context.astype(np.float32), alpha.astype(np.float32))
